# revision 21
# baseline (speedup 1.0000x reference)
"""ColBERT loss kernel for Trainium2, SPMD over 8 NeuronCores.

Problem: q [64,64,128], d_pos/d_neg [64,512,128], mask_pos/neg [64,512] ->
scalar CE loss over maxsim scores [64, 128].

Strategy (v2 — dual-port PSUM drain):
- Shard the 128 docs (64 pos + 64 neg) across 8 cores: 16 docs/core, q
  replicated.  The tiny [64,128] softmax+CE epilogue runs on host in f64.
- Mask folding on host: invalid doc tokens are replaced by that doc's
  token 0 (always valid per setup) and compacted to the front, so a plain
  max over a prefix of columns == masked max, exactly.  Docs sorted by
  valid-count descending per core (SPMD: one program for all cores).
- PE: one fp16 matmul per (query-pair, doc): sim = qT_p.T @ dT_d -> PSUM
  [128, fd] fp32.  128 partitions = 2 queries x 64 query tokens.
- The max over doc tokens is PSUM-drain-bound (DVE tensor_reduce is the
  only PSUM maxer and runs 1 elem/cycle @0.96GHz => ~157us baseline).
  v2 splits the drain across BOTH PSUM-read ports:
    * per 4-doc batch, NS[b] "staged" docs: ACT copies PSUM->SBUF fp16
      (1/cycle @1.2GHz, casting), then the max runs as a fp16
      tensor_tensor fold tree on DVE at 2x_1P (2 out/cycle) with the
      first fold level optionally on GPSIMD; a final 1x tensor_reduce
      eats the 36-wide tail.
    * the remaining docs: direct DVE tensor_reduce from PSUM (ragged
      per-batch free dim = smallest docs, thanks to the sort).
  Staged docs use a uniform fd=2*TH so fold APs batch a whole pair's
  staged docs (amortizes DVE op overheads); padding columns are copies
  of token 0 so the max is unaffected.
- Sum over the 64 query tokens via PE matmul of fp16 maxvals against a
  0/1 ones2 matrix: scores [16 docs, 2 queries] per 8-pair group.
"""

import numpy as np

import concourse.bass as bass
import concourse.mybir as mybir
import concourse.tile as tile
from concourse.bass_utils import run_bass_kernel_spmd
from concourse.vector_clock import ScopedClock

B, SQ, SD, H = 64, 64, 512, 128
NCORES = 8
DOCS_PER_CORE = 16  # 8 pos + 8 neg
NPAIRS = B // 2  # 32 query pairs
PAIR_GROUPS = 4  # 8 pairs per sum-matmul group
RB = 4  # docs per PSUM buffer (4 banks; 2 buffers = full PSUM)
NBATCH = DOCS_PER_CORE // RB  # 4
NS_DEFAULT = (4, 4, 4, 0)  # staged docs per batch (ACT path); rest direct DVE
L1_GPSIMD = False  # walrus rejects TensorTensor max on Pool (no Q7 ucode)
# Single 8-bank PSUM ring tile instead of rotating pools: enables 4-doc ACT
# staging ops (3/pair instead of 4) but any slot schedule on 2 bank-sets
# couples consecutive ACT reads to PE writes (sim: no net win, DVE becomes
# the cap).  Kept for reference; the tri-tile pools below measured best.
RING = False


def _patch_tile_drain():
    """walrus rejects >1 sync-wait on a Drain (CTRL) instruction; the
    TileContext tail drain carries one wait per outstanding semaphore.
    Split them across a chain of single-wait drains."""
    if getattr(tile.TileContext, "_drain_patched", False):
        return

    def _drain_and_barrier(self, tick_clock, wait_clock):
        nc = self.nc
        drain_inst = nc.sync.drain()
        wait_clock.add_sem_waits(
            drain_inst.ins, ScopedClock({None: tick_clock.global_clock})
        )
        si = drain_inst.ins.sync_info
        conds = list(si.on_wait) if (si is not None and si.on_wait) else []
        if len(conds) > 1:
            upd = list(si.on_update) if (si is not None and si.on_update) else []
            drain_inst.ins.sync_info = mybir.SyncInfo(on_wait=conds[:1], on_update=upd)
            for c in conds[1:]:
                extra = nc.sync.drain()
                extra.ins.sync_info = mybir.SyncInfo(on_wait=[c], on_update=[])
        nc.all_engine_barrier()
        assert self.sems is not None
        popped = nc._tile_sem_poison_stack.pop()
        assert popped is self._sem_poison
        nc.clear_and_free_semaphores(list(self.sems.allocated().values()))
        nc.all_engine_barrier()

    tile.TileContext._drain_and_barrier = _drain_and_barrier
    tile.TileContext._drain_patched = True


def _prune_self_waits(nc):
    """Drop sync-waits on an engine's own completion semaphore: engines
    execute their queue in order, so such waits are always satisfied at
    dispatch -- they only cost sequencer time."""
    for f in nc.m.functions:
        for blk in f.blocks:
            for inst in blk.instructions:
                si = inst.sync_info
                if si is None or not si.on_wait:
                    continue
                if inst.engine.name not in ("DVE", "Activation", "Pool"):
                    continue
                pref = f"{inst.engine.name}_"
                keep = [
                    c
                    for c in si.on_wait
                    if not (getattr(c, "ant_name", "") or "").startswith(pref)
                ]
                if len(keep) != len(si.on_wait):
                    inst.sync_info = mybir.SyncInfo(
                        on_wait=keep, on_update=list(si.on_update or [])
                    )


def _split_multi_waits(nc, max_waits=1):
    """This walrus build accepts at most one sync-wait per instruction.
    Hoist extra waits onto same-engine NoOps inserted just before."""
    for f in nc.m.functions:
        for blk in f.blocks:
            new = []
            changed = False
            for inst in blk.instructions:
                si = inst.sync_info
                conds = list(si.on_wait) if (si is not None and si.on_wait) else []
                if len(conds) > max_waits:
                    upd = list(si.on_update) if si.on_update else []
                    for c in conds[:-max_waits]:
                        nop = mybir.InstNoOp(name=f"I-wsplit-{nc.next_id()}")
                        nop.engine = inst.engine
                        nop.sync_info = mybir.SyncInfo(on_wait=[c], on_update=[])
                        new.append(nop)
                    inst.sync_info = mybir.SyncInfo(
                        on_wait=conds[-max_waits:], on_update=upd
                    )
                    changed = True
                new.append(inst)
            if changed:
                blk.instructions = new


def _build_program(loop_repeat=1, batch_ths=None):
    """batch_ths: (TH, NS, FDdir) from _host_prep:
    TH    -- uniform staged half-width (multiple of 8); staged fd = 2*TH
    NS    -- per-batch staged-doc count (docs j < NS[b] staged, rest direct)
    FDdir -- per-batch direct-doc matmul/reduce free dim (even)
    """
    TH, NS, FDdir = batch_ths
    assert TH % 8 == 0
    NSTG = sum(NS)
    _patch_tile_drain()
    f32 = mybir.dt.float32
    f16 = mybir.dt.float16
    alu_max = mybir.AluOpType.max
    nc = bass.Bass("TRN2", target_bir_lowering=False, debug=False, num_devices=NCORES)

    qh = nc.dram_tensor("qh", [H, B * SQ], f16, kind="ExternalInput").ap()
    dh = nc.dram_tensor("dh", [H, DOCS_PER_CORE * SD], f16, kind="ExternalInput").ap()
    ones2 = nc.dram_tensor("ones2", [H, 2], f16, kind="ExternalInput").ap()
    out = nc.dram_tensor(
        "scores_raw", [H, PAIR_GROUPS * 2], f32, kind="ExternalOutput"
    ).ap()

    import contextlib

    with tile.TileContext(nc) as tc, contextlib.ExitStack() as es:
        const_pool = es.enter_context(tc.tile_pool(name="const", bufs=1))
        # Staged sims: tri-batch tiles (3 docs x 1 bank) double-buffered =
        # 6 banks; direct sims: 4 docs x half-bank single tile = 2 banks.
        # Separate pools decouple the PE->ACT staging chain from the direct
        # DVE drain (with one 4-batch pool the rotation serialized PE on
        # ACT's last staging op each pair: ~680ns/pair bubble).
        if RING:
            ring_pool = es.enter_context(tc.tile_pool(name="ring", bufs=1, space="PSUM"))
        else:
            mm_pool = es.enter_context(tc.tile_pool(name="mm", bufs=2, space="PSUM"))
            dir_pool = es.enter_context(
                tc.tile_pool(name="dir", bufs=1, space="PSUM")
            )
        stage_pool = es.enter_context(tc.tile_pool(name="stage", bufs=2))
        f1_pool = es.enter_context(tc.tile_pool(name="f1", bufs=2))
        f2_pool = es.enter_context(tc.tile_pool(name="f2", bufs=2))
        f3_pool = es.enter_context(tc.tile_pool(name="f3", bufs=2))
        sb_pool = es.enter_context(tc.tile_pool(name="sb", bufs=1))

        qh_sb = const_pool.tile([H, B * SQ], f16)
        nc.sync.dma_start(qh_sb[:], qh[:])
        dh_sb = const_pool.tile([H, DOCS_PER_CORE * SD], f16)
        nc.sync.dma_start(dh_sb[:], dh[:])
        ones2_sb = const_pool.tile([H, 2], f16)
        nc.sync.dma_start(ones2_sb[:], ones2[:])

        maxvals = sb_pool.tile([H, NPAIRS, DOCS_PER_CORE], f16)
        scores_sb = sb_pool.tile([H, PAIR_GROUPS, 2], f32)

        # maxvals slots: 0:NSTG = staged docs (= docs 0:12 for NS=(4,4,4,0)),
        # NSTG:16 = direct docs (batch 3 = the 4 smallest, thanks to the
        # count-descending sort).
        assert NS == (4, 4, 4, 0), "layout assumes 12 staged + 4 direct docs"
        FD_DIR = FDdir[3]
        assert RING or FD_DIR <= 256, "direct docs must fit half a PSUM bank"

        ring = ring_pool.tile([H, 8, SD], f32, name="ring") if RING else None

        def emit_mm_ring(p):
            """Ring variant: one 8-bank PSUM tile, slots = banks.  Staged
            quads t=0,1,2 use slots (4t)%8..+4 (bank-aligned, so one 4-doc
            ACT op per quad: 3 ops/pair instead of 4); direct docs reuse
            slots 4..8 afterwards.  Tile range overlaps give the WAR deps a
            rotating pool would."""
            qslice = slice(p * 128, (p + 1) * 128)
            stage = stage_pool.tile([H, NSTG, 2 * TH], f16, name="stage")

            def quad(t, s0):
                for j in range(4):
                    d = 4 * t + j
                    nc.tensor.matmul(
                        ring[:, s0 + j, 0 : 2 * TH],
                        lhsT=qh_sb[:, qslice],
                        rhs=dh_sb[:, d * SD : d * SD + 2 * TH],
                        start=True,
                        stop=True,
                    )
                nc.scalar.copy(
                    stage[:, 4 * t : 4 * t + 4, :], ring[:, s0 : s0 + 4, 0 : 2 * TH]
                )

            def direct(s0):
                for j in range(RB):
                    d = 12 + j
                    nc.tensor.matmul(
                        ring[:, s0 + j, 0:FD_DIR],
                        lhsT=qh_sb[:, qslice],
                        rhs=dh_sb[:, d * SD : d * SD + FD_DIR],
                        start=True,
                        stop=True,
                    )
                nc.vector.tensor_reduce(
                    out=maxvals[:, p, NSTG:DOCS_PER_CORE],
                    in_=ring[:, s0 : s0 + 4, 0:FD_DIR],
                    axis=mybir.AxisListType.X,
                    op=alu_max,
                )

            # Slot schedule: the bank group a quad writes must have been
            # LAST read by an early-completing consumer two groups back.
            # t0@0-3 (2-back reader: direct's DVE reduce of prev pair),
            # t1@4-7 (reader: prev t2's ACT, already ordered before in the
            # ACT queue), dir@0-3 (reader: t0's ACT), t2@4-7 (t1's ACT).
            quad(0, 0)
            quad(1, 4)
            direct(0)
            quad(2, 4)
            return stage

        def emit_mm(p):
            """Matmuls + ACT staging + direct DVE reduces for pair p.
            Returns the stage tile for the fold chain."""
            qslice = slice(p * 128, (p + 1) * 128)
            stage = stage_pool.tile([H, NSTG, 2 * TH], f16, name="stage")
            for t in range(4):  # tri-batches of staged docs 3t..3t+2
                ps = mm_pool.tile([H, 3, SD], f32, tag="ps")
                for j in range(3):
                    d = 3 * t + j
                    nc.tensor.matmul(
                        ps[:, j, 0 : 2 * TH],
                        lhsT=qh_sb[:, qslice],
                        rhs=dh_sb[:, d * SD : d * SD + 2 * TH],
                        start=True,
                        stop=True,
                    )
                nc.scalar.copy(
                    stage[:, 3 * t : 3 * t + 3, :], ps[:, :, 0 : 2 * TH]
                )
            psd = dir_pool.tile([H, RB, 256], f32, tag="psd")
            for j in range(RB):
                d = 12 + j
                nc.tensor.matmul(
                    psd[:, j, 0:FD_DIR],
                    lhsT=qh_sb[:, qslice],
                    rhs=dh_sb[:, d * SD : d * SD + FD_DIR],
                    start=True,
                    stop=True,
                )
            nc.vector.tensor_reduce(
                out=maxvals[:, p, NSTG:DOCS_PER_CORE],
                in_=psd[:, :, 0:FD_DIR],
                axis=mybir.AxisListType.X,
                op=alu_max,
            )
            return stage

        def emit_folds(p, stage):
            if stage is None:
                return
            h1, h2, h3 = TH, TH // 2, TH // 4
            f1 = f1_pool.tile([H, NSTG, h1], f16)
            eng = nc.gpsimd if L1_GPSIMD else nc.vector
            eng.tensor_tensor(
                out=f1[:], in0=stage[:, :, 0:h1], in1=stage[:, :, h1 : 2 * h1],
                op=alu_max,
            )
            f2 = f2_pool.tile([H, NSTG, h2], f16)
            nc.vector.tensor_tensor(
                out=f2[:], in0=f1[:, :, 0:h2], in1=f1[:, :, h2:h1], op=alu_max
            )
            f3 = f3_pool.tile([H, NSTG, h3], f16)
            nc.vector.tensor_tensor(
                out=f3[:], in0=f2[:, :, 0:h3], in1=f2[:, :, h3:h2], op=alu_max
            )
            nc.vector.tensor_reduce(
                out=maxvals[:, p, 0:NSTG],
                in_=f3[:],
                axis=mybir.AxisListType.X,
                op=alu_max,
            )

        def emit_sums(g):
            if RING:
                sums = ring[:, g, :]
            else:
                sums_t = dir_pool.tile([H, RB, 256], f32, tag="psd", name="sums_t")
                sums = sums_t[:, 0, :]
            nc.tensor.matmul(
                sums[:, 0:2],
                lhsT=maxvals[:, g * 8 : (g + 1) * 8, :],
                rhs=ones2_sb[:],
                start=True,
                stop=True,
            )
            # ACT copy: keeps the end-of-body sums chain off the DVE queue,
            # which is still draining the last pair's folds.
            nc.scalar.copy(scores_sb[:, g, :], sums[:, 0:2])

        def body(_iv=None):
            mm = emit_mm_ring if RING else emit_mm
            prev = None
            for p in range(NPAIRS):
                if prev is not None:
                    emit_folds(p - 1, prev)
                prev = mm(p)
            emit_folds(NPAIRS - 1, prev)
            # All sum-matmuls at the end: their PSUM-pool slots + maxvals
            # reads would otherwise stall the matmul pipeline at every
            # group boundary.
            for g in range(PAIR_GROUPS):
                emit_sums(g)

        if loop_repeat > 1:
            with tc.For_i(0, loop_repeat, 1):
                body()
        else:
            body()

        nc.sync.dma_start(out[:], scores_sb[:, :, :])

    _prune_self_waits(nc)
    _split_multi_waits(nc)
    return nc


_PROGRAMS = {}


def _get_program(batch_ths):
    key = batch_ths
    if key not in _PROGRAMS:
        _PROGRAMS[key] = _build_program(batch_ths=batch_ths)
    return _PROGRAMS[key]


def _host_prep(q, d_pos, d_neg, mask_pos, mask_neg):
    q = np.asarray(q, dtype=np.float32)
    d_pos = np.asarray(d_pos, dtype=np.float32)
    d_neg = np.asarray(d_neg, dtype=np.float32)
    mask_pos = np.asarray(mask_pos)
    mask_neg = np.asarray(mask_neg)

    # Compact: move each doc's valid tokens to the front, pad the tail
    # with copies of token 0 (always valid per setup).  Plain max over
    # the first t columns == masked max, exactly.
    def compact(d, mask):
        out = np.empty_like(d)
        for b in range(d.shape[0]):
            v = d[b, mask[b] != 0]
            out[b, : len(v)] = v
            out[b, len(v) :] = d[b, 0]
        return out

    dp = compact(d_pos, mask_pos)
    dn = compact(d_neg, mask_neg)
    cp = mask_pos.sum(1)
    cn = mask_neg.sum(1)
    # Per core: sort its 16 docs by valid-count descending so batches hold
    # similar counts; direct docs (largest j per batch) get the smallest
    # counts -> small ragged FDdir.
    perms = []
    sorted_counts = np.zeros((NCORES, DOCS_PER_CORE), np.int64)
    for c in range(NCORES):
        counts = np.concatenate([cp[8 * c : 8 * c + 8], cn[8 * c : 8 * c + 8]])
        perm = np.argsort(-counts, kind="stable")
        perms.append(perm)
        sorted_counts[c] = counts[perm]

    NS = NS_DEFAULT
    TH = (int(sorted_counts.max()) + 1) // 2
    TH = min(SD // 2, (TH + 7) // 8 * 8)
    fdd = []
    for b in range(NBATCH):
        if NS[b] < RB:
            m = int(sorted_counts[:, b * RB + NS[b] : (b + 1) * RB].max())
            fdd.append(min(SD, (m + 1) // 2 * 2))
        else:
            fdd.append(2)
    cfg = (TH, tuple(NS), tuple(fdd))

    # qT[h, q*SQ + s], fp16 single pass
    qT = np.ascontiguousarray(q.transpose(2, 0, 1).reshape(H, B * SQ))
    qh = qT.astype(np.float16)
    dpT = dp.transpose(2, 0, 1)  # [H, 64, 512]
    dnT = dn.transpose(2, 0, 1)

    ones2 = np.zeros((H, 2), np.float16)
    ones2[:SQ, 0] = 1.0
    ones2[SQ:, 1] = 1.0

    in_maps = []
    for c in range(NCORES):
        dT_c = np.ascontiguousarray(
            np.concatenate(
                [dpT[:, 8 * c : 8 * c + 8, :], dnT[:, 8 * c : 8 * c + 8, :]], axis=1
            )[:, perms[c], :].reshape(H, DOCS_PER_CORE * SD)
        )
        in_maps.append(
            {
                "qh": qh,
                "dh": dT_c.astype(np.float16),
                "ones2": ones2,
            }
        )
    return in_maps, cfg, perms


def _host_epilogue(results, perms):
    # scores_raw rows: partition = pg*16 + d_local; cols: g*2 + j
    # query = 2*(8*g + pg) + j ; doc_local d: 0-7 pos docs 8c+d, 8-15 neg.
    dist = np.zeros((B, 2 * B), np.float32)
    for c in range(NCORES):
        arr = np.asarray(results[c]["scores_raw"])  # [128, 8]
        arr = arr.reshape(8, 16, PAIR_GROUPS, 2)  # [pg, d, g, j]
        s_qd = arr.transpose(2, 0, 3, 1).reshape(B, DOCS_PER_CORE)  # [query, slot]
        inv = np.empty_like(perms[c])
        inv[perms[c]] = np.arange(DOCS_PER_CORE)
        s_qd = s_qd[:, inv]  # [query, original local doc]
        dist[:, 8 * c : 8 * c + 8] = s_qd[:, 0:8]
        dist[:, B + 8 * c : B + 8 * c + 8] = s_qd[:, 8:16]

    d64 = dist.astype(np.float64)
    m = d64.max(axis=1, keepdims=True)
    logz = np.log(np.exp(d64 - m).sum(axis=1)) + m[:, 0]
    lbl = np.arange(B)
    loss = -(d64[lbl, lbl] - logz).mean()
    return np.array(loss, dtype=np.float32)


def kernel(q, d_pos, d_neg, mask_pos, mask_neg):
    in_maps, cfg, perms = _host_prep(q, d_pos, d_neg, mask_pos, mask_neg)
    nc = _get_program(cfg)
    last = None
    for _ in range(3):  # axon RPC occasionally throws transient INTERNAL errors
        try:
            res = run_bass_kernel_spmd(nc, in_maps, list(range(NCORES)), trace=False)
            return _host_epilogue(res.results, perms)
        except Exception as e:
            last = e
    raise last
